# revision 5
# baseline (speedup 1.0000x reference)
"""Grouped fp8 block-quantized GEMM (DeepSeekV3 GroupColumnParallelLinear) on 8 trn2 cores.

Math per group g (G=8, T=1024, K=7168, N=2048, BLOCK=128):
  a_scale[t,kb] = max|x[t, kb*128:(kb+1)*128]| / 448
  x_deq = fp8_e4m3fn_rne(x / a_scale) * a_scale
  w_deq = weight * scale (per 128x128 block)
  y = x_deq @ w_deq.T + bias     (fp32 accumulation)

Sharding: one group per NeuronCore (expert parallel, zero communication).

Host prep (layout only + folding the per-block scale into the stored weight):
  - w_deq precomputed in fp32, rounded once to bf16, laid out
    [56 kb][16 nt][128 k][128 n] so each (kb, nt) tile is k-partition-major.
  - bias laid out [128, 16] (per-partition vector per n-tile).

Device kernel per core (v2 - no PE transposes, single W stream):
  - act quant in natural t-major layout: absmax-reduce per (t, kb),
    m = 224/absmax, q = trn_fp8e4_rne(x*m), x_deq = q*(absmax/224) in bf16
    (TRN e4m3 max is 240 -> half-grid encoding, same as reference e4m3fn/448).
  - x_deq blocks are round-tripped through a DRAM scratch slab and pulled
    back with the X-bar DMA transpose (HWDGE, 2-byte dtype) to k-major
    [128 k, 1024 t] tiles -- no tensor-engine transposes at all.
  - K (56 kb) is processed in chunks; for each chunk a window of matmuls
    accumulates psum[n_tile, t_half] over the chunk's kb for all 16 n-tiles
    (4 segments x 8 psum banks), then drains add into an SBUF fp32
    accumulator. Quantization + round-trip of chunk c+1 overlaps window c,
    so DMA and vector work spread across the whole kernel.
  - W is streamed exactly once (29 MB); bias is folded into the first
    window's drain; the last window's drain writes y directly.
"""

import os
import sys

import numpy as np

for _p in ("/opt/trn_rl_repo",):
    if _p not in sys.path and os.path.isdir(_p):
        sys.path.insert(0, _p)

import ml_dtypes  # noqa: E402

G, T, K, N = 8, 1024, 7168, 2048
P = 128
KB = K // P  # 56
NT = N // P  # 16
TT = T // P  # 8
FP8_MAX = 448.0
HALF_MAX = 224.0  # TRN fp8e4 grid is e4m3fn/2 in our encoding

# kb-chunk ladder: first chunk small so the matmul windows start early;
# later chunks sized so quant of chunk c+1 fits inside window c.
CHUNKS = [8, 12, 12, 12, 12]
assert sum(CHUNKS) == KB
CMAX = max(CHUNKS)
NW = len(CHUNKS)

_NC_CACHE = {}


def _build_nc():
    import concourse.bacc as bacc
    import concourse.mybir as mybir
    import concourse.tile as tile

    dt = mybir.dt
    nc = bacc.Bacc("TRN2", target_bir_lowering=False, debug=False)

    x_d = nc.dram_tensor("x", [T, K], dt.float32, kind="ExternalInput")
    w_d = nc.dram_tensor("w", [KB, NT, P, P], dt.bfloat16, kind="ExternalInput")
    b_d = nc.dram_tensor("b", [P, NT], dt.float32, kind="ExternalInput")
    y_d = nc.dram_tensor("y", [N, T], dt.float32, kind="ExternalOutput")
    # internal DRAM scratch for the x_deq round-trip (kb-slab-major so each
    # kb slab [1024 t, 128 k] is contiguous for the X-bar transpose read)
    xh_d = nc.dram_tensor("xh", [KB, T, P], dt.bfloat16, kind="Internal")

    OP = mybir.AluOpType
    TH = T // 2  # 512, psum free dim

    chunk0 = [0]
    for c in CHUNKS[:-1]:
        chunk0.append(chunk0[-1] + c)

    with tile.TileContext(nc) as tc:
        with (
            tc.tile_pool(name="const", bufs=1) as const,
            tc.tile_pool(name="acc", bufs=1) as acc_p,
            tc.tile_pool(name="xin", bufs=2) as xin_p,
            tc.tile_pool(name="stats", bufs=4) as st_p,
            tc.tile_pool(name="xq8", bufs=2) as xq8_p,
            tc.tile_pool(name="xdq", bufs=2) as xdq_p,
            tc.tile_pool(name="xT", bufs=2) as xT_p,
            tc.tile_pool(name="wsb", bufs=2) as wsb_p,
            tc.tile_pool(name="ysb", bufs=4) as ysb_p,
            tc.tile_pool(name="tpsum", bufs=8, space="PSUM") as tps_p,
        ):
            bias_sb = const.tile([P, NT], dt.float32)
            nc.sync.dma_start(bias_sb[:], b_d[:, :])

            # PE warmup source (keeps the HAM clock-gate at 8/8 during the
            # quant head); dep-free dummy matmuls
            warm_w = const.tile([P, P], dt.bfloat16)
            nc.vector.memset(warm_w[:], 0.0)
            warm_src = const.tile([P, TH], dt.bfloat16)
            nc.vector.memset(warm_src[:], 0.0)

            # fp32 accumulator for the windowed contraction: acc[nt] = [128 n, 1024 t]
            acc = [acc_p.tile([P, T], dt.float32, name=f"acc{nt}") for nt in range(NT)]

            def quant_tt(c, tt, deng):
                """Quantize x[t-tile tt, chunk c] and write bf16 x_deq blocks
                to the DRAM scratch slab. All ops in natural t-major layout."""
                kb0, nkb = chunk0[c], CHUNKS[c]
                xin = xin_p.tile([P, CMAX, P], dt.float32, name="xin")
                nc.sync.dma_start(
                    xin[:, 0:nkb, :],
                    x_d[
                        tt * P : (tt + 1) * P, kb0 * P : (kb0 + nkb) * P
                    ].rearrange("p (a b) -> p a b", b=P),
                )
                amax = st_p.tile([P, CMAX], dt.float32, name="amax")
                m = st_p.tile([P, CMAX], dt.float32, name="m")
                a2 = st_p.tile([P, CMAX], dt.float32, name="a2")
                nc.vector.tensor_reduce(
                    amax[:, 0:nkb],
                    xin[:, 0:nkb, :],
                    axis=mybir.AxisListType.X,
                    op=OP.max,
                    apply_absolute_value=True,
                )
                nc.vector.reciprocal(m[:, 0:nkb], amax[:, 0:nkb])
                nc.vector.tensor_scalar_mul(m[:, 0:nkb], m[:, 0:nkb], HALF_MAX)
                nc.vector.tensor_scalar_mul(a2[:, 0:nkb], amax[:, 0:nkb], 1.0 / HALF_MAX)
                xq8 = xq8_p.tile([P, CMAX, P], dt.float8e4, name="xq8")
                nc.gpsimd.tensor_tensor(
                    xq8[:, 0:nkb, :],
                    xin[:, 0:nkb, :],
                    m[:, 0:nkb, None].to_broadcast((P, nkb, P)),
                    OP.mult,
                )
                xdq = xdq_p.tile([P, CMAX, P], dt.bfloat16, name="xdq")
                deng.tensor_tensor(
                    xdq[:, 0:nkb, :],
                    xq8[:, 0:nkb, :],
                    a2[:, 0:nkb, None].to_broadcast((P, nkb, P)),
                    OP.mult,
                )
                for j in range(nkb):
                    nc.sync.dma_start(
                        xh_d[kb0 + j, tt * P : (tt + 1) * P, :], xdq[:, j, :]
                    )

            def xT_load(c):
                """Pull chunk c back k-major via X-bar DMA transpose."""
                kb0, nkb = chunk0[c], CHUNKS[c]
                xTt = xT_p.tile([P, CMAX, T], dt.bfloat16, name="xT")
                for j in range(nkb):
                    nc.scalar.dma_start(
                        xTt[:, j, :], xh_d[kb0 + j, :, :], transpose=True
                    )
                return xTt

            def window(c, xTt):
                """All matmuls for kb-chunk c: 4 segments of 4 n-tiles, each
                segment accumulates psum[128 n, 512 t] x 8 banks over the
                chunk, then drains into acc (or bias-init / y-evict)."""
                kb0, nkb = chunk0[c], CHUNKS[c]
                for s in range(4):
                    wsb = wsb_p.tile([P, CMAX, 4, P], dt.bfloat16, name="wsb")
                    for nti in range(4):
                        nc.scalar.dma_start(
                            wsb[:, 0:nkb, nti, :],
                            w_d[kb0 : kb0 + nkb, s * 4 + nti, :, :].rearrange(
                                "a k n -> k a n"
                            ),
                        )
                    ps = [
                        tps_p.tile([P, TH], dt.float32, name="mpsum")
                        for _ in range(8)
                    ]
                    if c == 0 and s == 0:
                        # warmup: fill the quant head so HAM stays at 8/8
                        for _ in range(90):
                            nc.tensor.matmul(
                                ps[0][:], warm_w[:], warm_src[:],
                                start=True, stop=True,
                            )
                    for j in range(nkb):
                        for nti in range(4):
                            for th in range(2):
                                nc.tensor.matmul(
                                    ps[nti * 2 + th][:],
                                    wsb[:, j, nti, :],
                                    xTt[:, j, th * TH : (th + 1) * TH],
                                    start=(j == 0),
                                    stop=(j == nkb - 1),
                                )
                    # quant of chunk c+1 overlaps this window (2 t-tiles per
                    # segment), emitted before the drains so the DVE work
                    # runs while the PE is still on this segment's matmuls
                    if c + 1 < NW:
                        quant_tt(c + 1, 2 * s, nc.vector if s % 2 == 0 else nc.gpsimd)
                        quant_tt(c + 1, 2 * s + 1, nc.vector if s % 2 == 1 else nc.gpsimd)
                    for nti in range(4):
                        nt = s * 4 + nti
                        for th in range(2):
                            p = ps[nti * 2 + th]
                            asl = acc[nt][:, th * TH : (th + 1) * TH]
                            if c == 0:
                                nc.vector.tensor_tensor(
                                    asl,
                                    p[:],
                                    bias_sb[:, nt : nt + 1].to_broadcast((P, TH)),
                                    OP.add,
                                )
                            elif c == NW - 1:
                                y = ysb_p.tile([P, TH], dt.float32, name="ysb")
                                nc.vector.tensor_tensor(y[:], p[:], asl, OP.add)
                                nc.sync.dma_start(
                                    y_d[
                                        nt * P : (nt + 1) * P,
                                        th * TH : (th + 1) * TH,
                                    ],
                                    y[:],
                                )
                            else:
                                nc.vector.tensor_tensor(asl, p[:], asl, OP.add)

            # head: quantize + transpose chunk 0, then run the windows
            for tt in range(TT):
                quant_tt(0, tt, nc.vector if tt % 2 == 0 else nc.gpsimd)
            xTt = xT_load(0)
            for c in range(NW):
                window(c, xTt)  # emits quant of chunk c+1 inside
                if c + 1 < NW:
                    xTt = xT_load(c + 1)

    nc.compile()
    return nc


def _get_nc():
    if "nc" not in _NC_CACHE:
        _NC_CACHE["nc"] = _build_nc()
    return _NC_CACHE["nc"]


def _prep_inputs(xs, weight, scale, bias):
    bf16 = ml_dtypes.bfloat16
    in_maps = []
    for g in range(G):
        # fold per-block scale into the fp8 code values (exact fp32 mul of the
        # stored params), round once to the bf16 matmul operand precision
        w_deq = (
            weight[g].reshape(NT, P, KB, P)
            * scale[g].astype(np.float32)[:, None, :, None]
        ).astype(bf16)
        # [nt, n1, kb, k1] -> [kb, nt, k1, n1]  (k-partition-major tiles)
        w_host = np.ascontiguousarray(w_deq.transpose(2, 0, 3, 1))
        b_host = np.ascontiguousarray(bias[g].reshape(NT, P).T.astype(np.float32))
        in_maps.append(
            {
                "x": np.ascontiguousarray(xs[g], dtype=np.float32),
                "w": w_host,
                "b": b_host,
            }
        )
    return in_maps


def _install_ntff_shim():
    # this trimmed image lacks ``antenv.axon_hooks``; recreate it so
    # run_bass_kernel_spmd(trace=True) can reach the axon NTFF profiler
    import types

    if "antenv.axon_hooks" in sys.modules:
        return
    try:
        if "/root/.axon_site" not in sys.path:
            sys.path.insert(0, "/root/.axon_site")
        from trn_agent_boot.trn_boot import _ntff_profile_via_ctypes

        hook = _ntff_profile_via_ctypes("/opt/axon/libaxon_pjrt.so")
    except Exception:
        hook = None
    mod = types.ModuleType("antenv.axon_hooks")
    mod._hook = hook
    mod.get_axon_ntff_profile_hook = lambda: mod._hook
    mod.set_axon_ntff_profile_hook = lambda h: setattr(mod, "_hook", h)
    sys.modules["antenv.axon_hooks"] = mod
    try:
        import antenv

        antenv.axon_hooks = mod
    except Exception:
        pass


def kernel(xs, weight, scale, bias, _trace=False, _tmpdir=None):
    from concourse.bass_utils import run_bass_kernel_spmd

    if _trace:
        _install_ntff_shim()

    nc = _get_nc()
    in_maps = _prep_inputs(xs, weight, scale, bias)
    res = run_bass_kernel_spmd(
        nc, in_maps, list(range(G)), trace=_trace, tmpdir=_tmpdir
    )
    out = np.stack([r["y"].T for r in res.results]).astype(np.float32)
    if _trace:
        kernel.last_results = res
    return out


# revision 10
# speedup vs baseline: 1.2270x; 1.2270x over previous
"""Grouped fp8 block-quantized GEMM (DeepSeekV3 GroupColumnParallelLinear) on 8 trn2 cores.

Math per group g (G=8, T=1024, K=7168, N=2048, BLOCK=128):
  a_scale[t,kb] = max|x[t, kb*128:(kb+1)*128]| / 448
  x_deq = fp8_e4m3fn_rne(x / a_scale) * a_scale
  w_deq = weight * scale (per 128x128 block)
  y = x_deq @ w_deq.T + bias     (fp32 accumulation)

Sharding: one group per NeuronCore (expert parallel, zero communication).

Host prep (layout only + folding the per-block scale into the stored weight):
  - w_deq precomputed in fp32, rounded once to bf16, laid out
    [56 kb][16 nt][128 k][128 n] so each (kb, nt) tile is k-partition-major.
  - bias laid out [128, 16] (per-partition vector per n-tile).

Device kernel per core (v2 - no PE transposes, single W stream):
  - act quant in natural t-major layout: absmax-reduce per (t, kb),
    m = 224/absmax, q = trn_fp8e4_rne(x*m), x_deq = q*(absmax/224) in bf16
    (TRN e4m3 max is 240 -> half-grid encoding, same as reference e4m3fn/448).
  - x_deq blocks are round-tripped through a DRAM scratch slab and pulled
    back with the X-bar DMA transpose (HWDGE, 2-byte dtype) to k-major
    [128 k, 1024 t] tiles -- no tensor-engine transposes at all.
  - K (56 kb) is processed in chunks; for each chunk a window of matmuls
    accumulates psum[n_tile, t_half] over the chunk's kb for all 16 n-tiles
    (4 segments x 8 psum banks), then drains add into an SBUF fp32
    accumulator. Quantization + round-trip of chunk c+1 overlaps window c,
    so DMA and vector work spread across the whole kernel.
  - W is streamed exactly once (29 MB); bias is folded into the first
    window's drain; the last window's drain writes y directly.
"""

import os
import sys

import numpy as np

for _p in ("/opt/trn_rl_repo",):
    if _p not in sys.path and os.path.isdir(_p):
        sys.path.insert(0, _p)

import ml_dtypes  # noqa: E402

G, T, K, N = 8, 1024, 7168, 2048
P = 128
KB = K // P  # 56
NT = N // P  # 16
TT = T // P  # 8
FP8_MAX = 448.0
HALF_MAX = 224.0  # TRN fp8e4 grid is e4m3fn/2 in our encoding

# kb-chunk ladder: first chunk small so the matmul windows start early;
# later chunks sized so quant of chunk c+1 fits inside window c.
CHUNKS = [4, 8, 12, 12, 12, 8]
assert sum(CHUNKS) == KB
CMAX = max(CHUNKS)
NW = len(CHUNKS)

_NC_CACHE = {}


def _build_nc():
    import concourse.bacc as bacc
    import concourse.mybir as mybir
    import concourse.tile as tile

    dt = mybir.dt
    nc = bacc.Bacc("TRN2", target_bir_lowering=False, debug=False)

    x_d = nc.dram_tensor("x", [T, K], dt.float32, kind="ExternalInput")
    w_d = nc.dram_tensor("w", [KB, NT, P, P], dt.bfloat16, kind="ExternalInput")
    b_d = nc.dram_tensor("b", [P, NT], dt.float32, kind="ExternalInput")
    y_d = nc.dram_tensor("y", [N, T], dt.float32, kind="ExternalOutput")
    # internal DRAM scratch for the x_deq round-trip (kb-slab-major so each
    # kb slab [1024 t, 128 k] is contiguous for the X-bar transpose read)
    xh_d = nc.dram_tensor("xh", [KB, T, P], dt.bfloat16, kind="Internal")

    OP = mybir.AluOpType
    TH = T // 2  # 512, psum free dim

    chunk0 = [0]
    for c in CHUNKS[:-1]:
        chunk0.append(chunk0[-1] + c)

    with tile.TileContext(nc) as tc:
        with (
            tc.tile_pool(name="const", bufs=1) as const,
            tc.tile_pool(name="acc", bufs=1) as acc_p,
            tc.tile_pool(name="xin", bufs=5) as xin_p,
            tc.tile_pool(name="stats", bufs=4) as st_p,
            tc.tile_pool(name="xq8", bufs=2) as xq8_p,
            tc.tile_pool(name="xdq", bufs=2) as xdq_p,
            tc.tile_pool(name="xT", bufs=2) as xT_p,
            tc.tile_pool(name="wsb", bufs=2) as wsb_p,
            tc.tile_pool(name="ysb", bufs=2) as ysb_p,
            tc.tile_pool(name="tpsum", bufs=8, space="PSUM") as tps_p,
        ):
            bias_sb = const.tile([P, NT], dt.float32)
            nc.sync.dma_start(bias_sb[:], b_d[:, :])

            # PE warmup source (keeps the HAM clock-gate at 8/8 during the
            # quant head); dep-free dummy matmuls
            warm_w = const.tile([P, P], dt.bfloat16)
            nc.vector.memset(warm_w[:], 0.0)
            warm_src = const.tile([P, TH], dt.bfloat16)
            nc.vector.memset(warm_src[:], 0.0)

            # fp32 accumulator for the windowed contraction: acc[nt] = [128 n, 1024 t]
            acc = [acc_p.tile([P, T], dt.float32, name=f"acc{nt}") for nt in range(NT)]

            def quant_load(c, tt):
                """Prefetch x[t-tile tt, chunk c] (emitted early so the SP
                HWDGE ring issues these ahead of the dependent compute)."""
                kb0, nkb = chunk0[c], CHUNKS[c]
                xin = xin_p.tile([P, CMAX, P], dt.float32, name="xin")
                nc.sync.dma_start(
                    xin[:, 0:nkb, :],
                    x_d[
                        tt * P : (tt + 1) * P, kb0 * P : (kb0 + nkb) * P
                    ].rearrange("p (a b) -> p a b", b=P),
                )
                return xin

            def quant_compute(c, tt, xin):
                """Quantize one t-tile of chunk c and write bf16 x_deq blocks
                to the DRAM scratch slab. DVE does only the short stats chain;
                the two big multiplies run on gpsimd so the DVE queue stays
                clear for psum drains."""
                kb0, nkb = chunk0[c], CHUNKS[c]
                amax = st_p.tile([P, CMAX], dt.float32, name="amax")
                m = st_p.tile([P, CMAX], dt.float32, name="m")
                a2 = st_p.tile([P, CMAX], dt.float32, name="a2")
                nc.vector.tensor_reduce(
                    amax[:, 0:nkb],
                    xin[:, 0:nkb, :],
                    axis=mybir.AxisListType.X,
                    op=OP.max,
                    apply_absolute_value=True,
                )
                nc.vector.reciprocal(m[:, 0:nkb], amax[:, 0:nkb])
                nc.vector.tensor_scalar_mul(m[:, 0:nkb], m[:, 0:nkb], HALF_MAX)
                nc.vector.tensor_scalar_mul(a2[:, 0:nkb], amax[:, 0:nkb], 1.0 / HALF_MAX)
                xq8 = xq8_p.tile([P, CMAX, P], dt.float8e4, name="xq8")
                nc.gpsimd.tensor_tensor(
                    xq8[:, 0:nkb, :],
                    xin[:, 0:nkb, :],
                    m[:, 0:nkb, None].to_broadcast((P, nkb, P)),
                    OP.mult,
                )
                xdq = xdq_p.tile([P, CMAX, P], dt.bfloat16, name="xdq")
                nc.gpsimd.tensor_tensor(
                    xdq[:, 0:nkb, :],
                    xq8[:, 0:nkb, :],
                    a2[:, 0:nkb, None].to_broadcast((P, nkb, P)),
                    OP.mult,
                )
                # one merged DMA for all nkb blocks of this t-tile
                nc.sync.dma_start(
                    xh_d[kb0 : kb0 + nkb, tt * P : (tt + 1) * P, :].rearrange(
                        "a p b -> p a b"
                    ),
                    xdq[:, 0:nkb, :],
                )

            def xT_load(c):
                """Pull chunk c back k-major via X-bar DMA transpose."""
                kb0, nkb = chunk0[c], CHUNKS[c]
                xTt = xT_p.tile([P, CMAX, T], dt.bfloat16, name="xT")
                for j in range(nkb):
                    nc.scalar.dma_start(
                        xTt[:, j, :], xh_d[kb0 + j, :, :], transpose=True
                    )
                return xTt

            def window(c, xTt):
                """All matmuls for kb-chunk c: 4 segments of 4 n-tiles, each
                segment accumulates psum[128 n, 512 t] x 8 banks over the
                chunk, then drains into acc (or bias-init / y-evict).
                Quantization of chunk c+1 is interleaved (2 t-tiles per
                segment, xin prefetched at window start)."""
                kb0, nkb = chunk0[c], CHUNKS[c]
                xins = []
                for s in range(4):
                    wsb = wsb_p.tile([P, CMAX, 4, P], dt.bfloat16, name="wsb")
                    for nti in range(4):
                        nc.scalar.dma_start(
                            wsb[:, 0:nkb, nti, :],
                            w_d[kb0 : kb0 + nkb, s * 4 + nti, :, :].rearrange(
                                "a k n -> k a n"
                            ),
                        )
                    if c + 1 < NW and s == 0:
                        # prefetch the whole next chunk's x input up front
                        xins = [quant_load(c + 1, tt) for tt in range(TT)]
                    ps = [
                        tps_p.tile([P, TH], dt.float32, name="mpsum")
                        for _ in range(8)
                    ]
                    if c == 0 and s == 0:
                        # warmup: fill the quant head so HAM stays at 8/8
                        for _ in range(90):
                            nc.tensor.matmul(
                                ps[0][:], warm_w[:], warm_src[:],
                                start=True, stop=True,
                            )
                    for j in range(nkb):
                        for nti in range(4):
                            for th in range(2):
                                nc.tensor.matmul(
                                    ps[nti * 2 + th][:],
                                    wsb[:, j, nti, :],
                                    xTt[:, j, th * TH : (th + 1) * TH],
                                    start=(j == 0),
                                    stop=(j == nkb - 1),
                                )
                    if c + 1 < NW:
                        quant_compute(c + 1, 2 * s, xins[2 * s])
                        quant_compute(c + 1, 2 * s + 1, xins[2 * s + 1])
                    for nti in range(4):
                        nt = s * 4 + nti
                        for th in range(2):
                            p = ps[nti * 2 + th]
                            asl = acc[nt][:, th * TH : (th + 1) * TH]
                            if c == 0:
                                nc.vector.tensor_tensor(
                                    asl,
                                    p[:],
                                    bias_sb[:, nt : nt + 1].to_broadcast((P, TH)),
                                    OP.add,
                                )
                            elif c == NW - 1:
                                y = ysb_p.tile([P, TH], dt.float32, name="ysb")
                                nc.vector.tensor_tensor(y[:], p[:], asl, OP.add)
                                nc.sync.dma_start(
                                    y_d[
                                        nt * P : (nt + 1) * P,
                                        th * TH : (th + 1) * TH,
                                    ],
                                    y[:],
                                )
                            else:
                                nc.vector.tensor_tensor(asl, p[:], asl, OP.add)

            # head: quantize + transpose chunk 0, then run the windows
            xins0 = [quant_load(0, tt) for tt in range(TT)]
            for tt in range(TT):
                quant_compute(0, tt, xins0[tt])
            xTt = xT_load(0)
            for c in range(NW):
                window(c, xTt)  # emits quant of chunk c+1 inside
                if c + 1 < NW:
                    xTt = xT_load(c + 1)

    nc.compile()
    return nc


def _get_nc():
    if "nc" not in _NC_CACHE:
        _NC_CACHE["nc"] = _build_nc()
    return _NC_CACHE["nc"]


def _prep_inputs(xs, weight, scale, bias):
    bf16 = ml_dtypes.bfloat16
    in_maps = []
    for g in range(G):
        # fold per-block scale into the fp8 code values (exact fp32 mul of the
        # stored params), round once to the bf16 matmul operand precision
        w_deq = (
            weight[g].reshape(NT, P, KB, P)
            * scale[g].astype(np.float32)[:, None, :, None]
        ).astype(bf16)
        # [nt, n1, kb, k1] -> [kb, nt, k1, n1]  (k-partition-major tiles)
        w_host = np.ascontiguousarray(w_deq.transpose(2, 0, 3, 1))
        b_host = np.ascontiguousarray(bias[g].reshape(NT, P).T.astype(np.float32))
        in_maps.append(
            {
                "x": np.ascontiguousarray(xs[g], dtype=np.float32),
                "w": w_host,
                "b": b_host,
            }
        )
    return in_maps


def _install_ntff_shim():
    # this trimmed image lacks ``antenv.axon_hooks``; recreate it so
    # run_bass_kernel_spmd(trace=True) can reach the axon NTFF profiler
    import types

    if "antenv.axon_hooks" in sys.modules:
        return
    try:
        if "/root/.axon_site" not in sys.path:
            sys.path.insert(0, "/root/.axon_site")
        from trn_agent_boot.trn_boot import _ntff_profile_via_ctypes

        hook = _ntff_profile_via_ctypes("/opt/axon/libaxon_pjrt.so")
    except Exception:
        hook = None
    mod = types.ModuleType("antenv.axon_hooks")
    mod._hook = hook
    mod.get_axon_ntff_profile_hook = lambda: mod._hook
    mod.set_axon_ntff_profile_hook = lambda h: setattr(mod, "_hook", h)
    sys.modules["antenv.axon_hooks"] = mod
    try:
        import antenv

        antenv.axon_hooks = mod
    except Exception:
        pass


def kernel(xs, weight, scale, bias, _trace=False, _tmpdir=None):
    from concourse.bass_utils import run_bass_kernel_spmd

    if _trace:
        _install_ntff_shim()

    nc = _get_nc()
    in_maps = _prep_inputs(xs, weight, scale, bias)
    res = run_bass_kernel_spmd(
        nc, in_maps, list(range(G)), trace=_trace, tmpdir=_tmpdir
    )
    out = np.stack([r["y"].T for r in res.results]).astype(np.float32)
    if _trace:
        kernel.last_results = res
    return out
